# revision 5
# baseline (speedup 1.0000x reference)
"""AttentionV1 Trainium2 Bass kernel, v2 (bf16 fused conv).

Data-parallel over batch: 8 images -> 8 NeuronCores. Per core:
  q,k = fused (1x1 conv + 3x3 depthwise) on PE bf16: contraction (c, tap)
        as 18 stationary passes per 128-out Mtile over padded x views.
  v   = 1x1 conv on PE bf16 + 9-tap stencil on DVE (STT chains, ~2x mode;
        v-ch 128:192 packed as two spatial halves across partitions)
  qf,kf = (conv_psum) * f  (DVE STT, writes bf16)
  norms = ACT Square accum_out; transposes = batched multi-chunk DMA xbar
  G = qf @ kf^T on PE over 128 n-chunks; softmax smalls; out = MT^T @ v.
"""
import sys

for _p in ("/opt/trn_rl_repo",):
    if _p not in sys.path:
        sys.path.insert(0, _p)

import numpy as np

import concourse.bass as bass
import concourse.bacc as bacc
import concourse.mybir as mybir
from concourse.tile import TileContext
from concourse.bass_utils import run_bass_kernel_spmd
from concourse.ap import AP

F32 = mybir.dt.float32
BF16 = mybir.dt.bfloat16
AL = mybir.AluOpType
AF = mybir.ActivationFunctionType

C = 192
O = 576
H = 128
W = 128
N = H * W
HEADS = 8
CH = 24
TR = 16            # rows per y-tile
NT = H // TR       # 8 tiles
TS = TR * W        # 2048 cols per tile
PW = 130           # padded pitch
NPT = 18 * PW      # per-tile padded grid (18 rows incl halo)

TAPD = [dy * PW + dx for dy in (-1, 0, 1) for dx in (-1, 0, 1)]
KT = [(0, 128), (128, 64)]

# q,k conv Mtiles (128-wide), f-kind: 0=f[0:128], 1=f[128:192]||f[0:64],
# 2=f[64:192]
MTS = [0, 1, 2]


def build_nc():
    nc = bacc.Bacc()
    x_d = nc.declare_dram_parameter("x", [C, H, W], F32, isOutput=False)
    f_d = nc.declare_dram_parameter("f", [C, H, W], F32, isOutput=False)
    # 54 stationaries [c-tile, 128 outs] bf16; idx = (mt*2+kt)*9+tap
    w3_d = nc.declare_dram_parameter("w3", [128, 54 * 128], BF16, isOutput=False)
    wv_d = nc.declare_dram_parameter("wv", [C, C], BF16, isOutput=False)
    tapsv_d = nc.declare_dram_parameter("tapsv", [128, 18], F32, isOutput=False)
    wp_d = nc.declare_dram_parameter("wp", [C, C], BF16, isOutput=False)
    temp_d = nc.declare_dram_parameter("temp", [CH, HEADS], F32, isOutput=False)
    id_d = nc.declare_dram_parameter("ident", [128, 128], F32, isOutput=False)
    out_d = nc.declare_dram_parameter("out", [C, H, W], F32, isOutput=True)

    with TileContext(nc) as tc:
        with (
            tc.tile_pool(name="const", bufs=1) as cpool,
            tc.tile_pool(name="xp", bufs=2) as xpool,
            tc.tile_pool(name="fin", bufs=2) as fpool,
            tc.tile_pool(name="qf", bufs=1) as qfpool,
            tc.tile_pool(name="scr", bufs=1) as scrpool,
            tc.tile_pool(name="tr", bufs=2) as trpool,
            tc.tile_pool(name="vbuf", bufs=1) as vpool,
            tc.tile_pool(name="vpad", bufs=2) as vppool,
            tc.tile_pool(name="fin2", bufs=1) as finpool,
            tc.tile_pool(name="outsb", bufs=2) as outpool,
            tc.tile_pool(name="cps", bufs=6, space="PSUM") as cpsum,
            tc.tile_pool(name="gps", bufs=1, space="PSUM") as gpsum,
        ):
            # ================= constants =================
            w3 = cpool.tile([128, 54 * 128], BF16, tag="w3", name="w3")
            nc.sync.dma_start(out=w3[:], in_=w3_d[:])
            wv_sb = [cpool.tile([128, C], BF16, tag="wv0", name="wv0"),
                     cpool.tile([64, C], BF16, tag="wv1", name="wv1")]
            nc.sync.dma_start(out=wv_sb[0][:], in_=wv_d[0:128, :])
            nc.sync.dma_start(out=wv_sb[1][:], in_=wv_d[128:192, :])
            tapsv = cpool.tile([128, 18], F32, tag="tapsv", name="tapsv")
            nc.sync.dma_start(out=tapsv[:], in_=tapsv_d[:])
            wp_sb = [cpool.tile([96, C], BF16, tag="wp0", name="wp0"),
                     cpool.tile([96, C], BF16, tag="wp1", name="wp1")]
            nc.sync.dma_start(out=wp_sb[0][:], in_=wp_d[0:96, :])
            nc.sync.dma_start(out=wp_sb[1][:], in_=wp_d[96:192, :])
            temp_sb = cpool.tile([CH, HEADS], F32, tag="temp", name="temp")
            nc.sync.dma_start(out=temp_sb[:], in_=temp_d[:])
            ident = cpool.tile([128, 128], F32, tag="ident", name="ident")
            nc.sync.dma_start(out=ident[:], in_=id_d[:])

            # split-set v-pad (v-ch 128:192 as two spatial halves), full image
            vsplit = vpool.tile([128, 66 * PW], BF16, tag="vsplit", name="vsplit")
            nc.vector.memset(vsplit[0:64, 0:PW], 0.0)
            nc.vector.memset(vsplit[64:128, 65 * PW:66 * PW], 0.0)
            vsp_cols = vsplit[:].rearrange("p (y x) -> p y x", x=PW)
            nc.vector.memset(vsp_cols[:, :, 0:1], 0.0)
            nc.vector.memset(vsp_cols[:, :, PW - 1:PW], 0.0)
            v_sb = vpool.tile([128, N], BF16, tag="vsb", name="vsb")
            v_sp = vpool.tile([128, N // 2], BF16, tag="vsp", name="vsp")

            sqacc = cpool.tile([128, 3 * NT], F32, tag="sqacc", name="sqacc")

            g_ps = [gpsum.tile([128, C], F32, tag="g0", name="g0"),
                    gpsum.tile([64, C], F32, tag="g1", name="g1")]

            def st_ap(mt, kt, j):
                idx = (mt * 2 + kt) * 9 + j
                ksz = KT[kt][1]
                return w3[0:ksz, idx * 128:(idx + 1) * 128]

            prev = {}

            for t in range(NT):
                _stage_tile(nc, t, x_d, f_d, xpool, fpool, vppool,
                            cpsum, wv_sb, vsplit, prev)
                _conv_tile(nc, t, prev, st_ap, cpsum, qfpool, trpool,
                           scrpool, sqacc, g_ps)
                if t >= 1:
                    _v_stencil(nc, t - 1, prev, v_sb, tapsv)
            _v_stencil(nc, NT - 1, prev, v_sb, tapsv)
            _v_stencil_split(nc, vsplit, v_sp, tapsv, scrpool)

            # ================= small phase =================
            rq = []
            for p in range(3):
                s1 = finpool.tile([128, 1], F32, tag=f"s1_{p}", name=f"s1_{p}")
                nc.vector.tensor_reduce(
                    s1[:], sqacc[:, p * NT:(p + 1) * NT],
                    axis=mybir.AxisListType.X, op=AL.add)
                nc.vector.tensor_scalar_max(s1[:], s1[:], 1e-24)
                nq = finpool.tile([128, 1], F32, tag=f"nq_{p}", name=f"nq_{p}")
                nc.scalar.activation(nq[:], s1[:], AF.Sqrt)
                r = finpool.tile([128, 1], F32, tag=f"rq_{p}", name=f"rq_{p}")
                nc.vector.reciprocal(r[:], nq[:])
                rq.append(r)

            G_sb = [finpool.tile([128, C], F32, tag="G0", name="G0"),
                    finpool.tile([64, C], F32, tag="G1", name="G1")]
            nc.vector.tensor_scalar_mul(G_sb[0][:], g_ps[0][:], rq[0][:])
            nc.vector.tensor_scalar_mul(G_sb[1][:], g_ps[1][:], rq[1][0:64, :])

            gt_ps = [cpsum.tile([128, 512], F32, tag="cps", name="gt0"),
                     cpsum.tile([128, 512], F32, tag="cps", name="gt1")]
            nc.tensor.matmul(gt_ps[0][0:128, 0:128], G_sb[0][:, 0:128], ident[:],
                             is_transpose=True, start=True, stop=True)
            nc.tensor.matmul(gt_ps[0][0:128, 128:192], G_sb[1][:, 0:128],
                             ident[0:64, 0:64], is_transpose=True,
                             start=True, stop=True)
            nc.tensor.matmul(gt_ps[1][0:64, 0:128], G_sb[0][:, 128:192], ident[:],
                             is_transpose=True, start=True, stop=True)
            nc.tensor.matmul(gt_ps[1][0:64, 128:192], G_sb[1][:, 128:192],
                             ident[0:64, 0:64], is_transpose=True,
                             start=True, stop=True)

            rk0 = finpool.tile([128, 1], F32, tag="rk0", name="rk0")
            nc.sync.dma_start(out=rk0[0:64, :], in_=rq[1][64:128, :])
            nc.sync.dma_start(out=rk0[64:128, :], in_=rq[2][0:64, :])
            rk1 = finpool.tile([64, 1], F32, tag="rk1", name="rk1")
            nc.sync.dma_start(out=rk1[:], in_=rq[2][64:128, :])

            gt_sb = [finpool.tile([128, C], F32, tag="gts0", name="gts0"),
                     finpool.tile([64, C], F32, tag="gts1", name="gts1")]
            nc.vector.tensor_scalar_mul(gt_sb[0][:], gt_ps[0][0:128, 0:C], rk0[:])
            nc.vector.tensor_scalar_mul(gt_sb[1][:], gt_ps[1][0:64, 0:C], rk1[:])

            at = finpool.tile([32, HEADS * 32], F32, tag="at", name="at")
            nc.vector.memset(at[:], 0.0)
            for h in range(HEADS):
                a0 = h * CH
                a1 = a0 + CH
                col = slice(a0, a1)
                if a1 <= 128:
                    nc.sync.dma_start(out=at[0:CH, h * 32:h * 32 + CH],
                                      in_=gt_sb[0][a0:a1, col])
                elif a0 >= 128:
                    nc.sync.dma_start(out=at[0:CH, h * 32:h * 32 + CH],
                                      in_=gt_sb[1][a0 - 128:a1 - 128, col])
                else:
                    m = 128 - a0
                    nc.sync.dma_start(out=at[0:m, h * 32:h * 32 + CH],
                                      in_=gt_sb[0][a0:128, col])
                    nc.sync.dma_start(out=at[m:CH, h * 32:h * 32 + CH],
                                      in_=gt_sb[1][0:a1 - 128, col])
                nc.vector.tensor_scalar_mul(
                    at[0:CH, h * 32:h * 32 + CH],
                    at[0:CH, h * 32:h * 32 + CH],
                    temp_sb[0:CH, h:h + 1])

            a_sb = finpool.tile([32, HEADS * 32], F32, tag="a", name="a")
            nc.vector.transpose(a_sb[:], at[:])
            e_sb = finpool.tile([32, HEADS * 32], F32, tag="e", name="e")
            nc.scalar.activation(e_sb[:], a_sb[:], AF.Exp)
            e3 = e_sb[:].rearrange("p (h d) -> p h d", d=32)
            sums = finpool.tile([CH, HEADS], F32, tag="sums", name="sums")
            nc.vector.tensor_reduce(
                sums[:], e3[0:CH, :, 0:CH], axis=mybir.AxisListType.X, op=AL.add)
            rs = finpool.tile([CH, HEADS], F32, tag="rs", name="rs")
            nc.vector.reciprocal(rs[:], sums[:])
            attn = finpool.tile([CH, HEADS * CH], BF16, tag="attn", name="attn")
            for h in range(HEADS):
                nc.vector.tensor_scalar_mul(
                    attn[:, h * CH:(h + 1) * CH],
                    e_sb[0:CH, h * 32:h * 32 + CH],
                    rs[:, h:h + 1])

            bd = [finpool.tile([96, C], BF16, tag="bd0", name="bd0"),
                  finpool.tile([96, C], BF16, tag="bd1", name="bd1")]
            nc.vector.memset(bd[0][:], 0.0)
            nc.vector.memset(bd[1][:], 0.0)
            for h in range(HEADS):
                nc.sync.dma_start(
                    out=bd[h // 4][(h % 4) * CH:(h % 4) * CH + CH,
                                   h * CH:(h + 1) * CH],
                    in_=attn[:, h * CH:(h + 1) * CH])
            mt_ps = [cpsum.tile([128, 512], F32, tag="cps", name="mt0"),
                     cpsum.tile([128, 512], F32, tag="cps", name="mt1")]
            for mi, msl in enumerate((slice(0, 128), slice(128, 192))):
                for k in range(2):
                    nc.tensor.matmul(mt_ps[mi][0:(128 if mi==0 else 64), 0:C], bd[k][:, msl], wp_sb[k][:],
                                     start=(k == 0), stop=(k == 1))
            mt_sb = [finpool.tile([128, C], BF16, tag="mt_sb0", name="mt_sb0"),
                     finpool.tile([128, C], BF16, tag="mt_sb1", name="mt_sb1")]
            nc.vector.tensor_copy(mt_sb[0][:], mt_ps[0][:, 0:C])
            nc.vector.tensor_copy(mt_sb[1][0:64, :], mt_ps[1][0:64, 0:C])
            nc.sync.dma_start(out=mt_sb[1][64:128, :], in_=mt_sb[1][0:64, :])

            # ---- output: out = MT^T @ v ----
            for j in range(N // 512):
                col = slice(j * 512, (j + 1) * 512)
                if j < 16:
                    vmv = v_sp[0:64, col]
                    mtl = mt_sb[1][0:64, :]
                else:
                    vmv = v_sp[64:128, (j - 16) * 512:(j - 15) * 512]
                    mtl = mt_sb[1][64:128, :]
                for mi, (msz, msl) in enumerate(((128, slice(0, 128)),
                                                 (64, slice(128, 192)))):
                    ps = cpsum.tile([128, 512], F32, tag="cps", name="ops")
                    nc.tensor.matmul(ps[0:msz, :], mt_sb[0][:, msl],
                                     v_sb[:, col], start=True, stop=False)
                    nc.tensor.matmul(ps[0:msz, :], mtl[:, msl],
                                     vmv, start=False, stop=True)
                    osb = outpool.tile([msz, 512], F32, tag=f"osb{mi}",
                                       name=f"osb{mi}")
                    nc.vector.tensor_copy(osb[:], ps[0:msz, :])
                    cs = 0 if mi == 0 else 128
                    nc.sync.dma_start(
                        out=out_d.rearrange("c h w -> c (h w)")[cs:cs + msz, col],
                        in_=osb[:])
    nc.finalize()
    return nc


def _stage_tile(nc, t, x_d, f_d, xpool, fpool, vppool, cpsum, wv_sb,
                vsplit, prev):
    """Load padded x tile (halo re-read from HBM), f tiles; v 1x1 + drains."""
    r0 = t * TR
    lo = r0 - 1
    hi = r0 + TR + 1
    dlo = max(lo, 0)
    dhi = min(hi, H)
    off = dlo - lo  # 1 if clipped at top
    xp = [xpool.tile([128, NPT], BF16, tag="xp0", name="xp0"),
          xpool.tile([64, NPT], BF16, tag="xp1", name="xp1")]
    prev.setdefault("xp", {})[t] = xp
    for ci, (cs, csz) in enumerate(KT):
        x3 = xp[ci][:].rearrange("p (y x) -> p y x", x=PW)
        nc.vector.memset(x3[:, :, 0:1], 0.0)
        nc.vector.memset(x3[:, :, PW - 1:PW], 0.0)
        if lo < 0:
            nc.vector.memset(x3[:, 0:1, :], 0.0)
        if hi > H:
            nc.vector.memset(x3[:, 17:18, :], 0.0)
        nc.gpsimd.dma_start(
            out=x3[:, off:off + dhi - dlo, 1:1 + W],
            in_=x_d[cs:cs + csz, dlo:dhi, :])

    fA = fpool.tile([128, TS], BF16, tag="fA", name="fA")
    fC = fpool.tile([128, TS], BF16, tag="fC", name="fC")
    fB = fpool.tile([128, TS], BF16, tag="fB", name="fB")
    nc.gpsimd.dma_start(out=fA[:], in_=f_d[0:128, r0:r0 + TR, :])
    nc.gpsimd.dma_start(out=fC[:], in_=f_d[64:192, r0:r0 + TR, :])
    nc.sync.dma_start(out=fB[0:64, :], in_=fC[64:128, :])
    nc.scalar.dma_start(out=fB[64:128, :], in_=fA[0:64, :])
    prev.setdefault("f", {})[t] = (fA, fB, fC)

    # ---- v 1x1 conv + drain into padded stencil buffers ----
    vp = vppool.tile([128, NPT], BF16, tag="vp", name="vp")
    prev.setdefault("vp", {})[t] = vp
    vp3 = vp[:].rearrange("p (y x) -> p y x", x=PW)
    nc.vector.memset(vp3[:, :, 0:1], 0.0)
    nc.vector.memset(vp3[:, :, PW - 1:PW], 0.0)
    if t == 0:
        nc.vector.memset(vp3[:, 0:1, :], 0.0)
    if t == NT - 1:
        nc.vector.memset(vp3[:, 17:18, :], 0.0)
    vsp3 = vsplit[:].rearrange("p (y x) -> p y x", x=PW)
    for wi in range(4):
        vt = [cpsum.tile([128, 512], F32, tag="cps", name="v0"),
              cpsum.tile([128, 512], F32, tag="cps", name="v1")]
        vps = [vt[0][:], vt[1][0:64, :]]
        rr = wi * 4
        for mi, (msz, mo) in enumerate(((128, 0), (64, 128))):
            for k, (cs, csz) in enumerate(KT):
                mvx = prev["xp"][t][k][:].rearrange(
                    "p (y x) -> p y x", x=PW)[:, 1 + rr:1 + rr + 4, 1:1 + W]
                nc.tensor.matmul(
                    vt[mi][0:msz, :], wv_sb[k][:, mo:mo + msz],
                    mvx, start=(k == 0), stop=(k == 1))
        nc.scalar.activation(
            vp3[:, 1 + rr:1 + rr + 4, 1:1 + W],
            vt[0][:].rearrange("p (y x) -> p y x", x=W), AF.Copy)
        if t < 4:
            dstp = vsp3[0:64, 1 + r0 + rr:1 + r0 + rr + 4, 1:1 + W]
        else:
            dstp = vsp3[64:128, 1 + (r0 - 64) + rr:1 + (r0 - 64) + rr + 4,
                        1:1 + W]
        nc.scalar.activation(
            dstp, vt[1][0:64, :].rearrange("p (y x) -> p y x", x=W), AF.Copy)
        for r in range(4):
            yy = r0 + rr + r
            if yy == 63:
                nc.scalar.activation(
                    vsp3[64:128, 0:1, 1:1 + W],
                    vt[1][0:64, r * W:(r + 1) * W].unsqueeze(1), AF.Copy)
            if yy == 64:
                nc.scalar.activation(
                    vsp3[0:64, 65:66, 1:1 + W],
                    vt[1][0:64, r * W:(r + 1) * W].unsqueeze(1), AF.Copy)
    # halo exchange for 128-set v-pad
    if t >= 1:
        pvp = prev["vp"][t - 1]
        nc.vector.tensor_copy(pvp[:, 17 * PW:18 * PW], vp[:, 1 * PW:2 * PW])
        nc.vector.tensor_copy(vp[:, 0:PW], pvp[:, 16 * PW:17 * PW])


def _conv_tile(nc, t, prev, st_ap, cpsum, qfpool, trpool, scrpool, sqacc,
               g_ps):
    """bf16 fused conv for q,k on tile t + drain + squares + transpose + gram."""
    xp = prev["xp"][t]
    ftiles = prev["f"][t]
    sptmp = prev.get("sq_scr")
    if sptmp is None:
        sptmp = scrpool.tile([128, TS], BF16, tag="sqscr", name="sqscr")
        prev["sq_scr"] = sptmp
    qkfT = trpool.tile([128, TR * 2 * C], BF16, tag="qkfT", name="qkfT")
    for mt in range(3):
        cps = [cpsum.tile([128, 512], F32, tag="cps", name="cps")
               for _ in range(4)]
        for kt in range(2):
            for j in range(9):
                st = st_ap(mt, kt, j)
                first = (kt == 0 and j == 0)
                last = (kt == 1 and j == 8)
                _conv_mms(nc, cps, xp[kt], st, j, first, last)
        qft = qfpool.tile([128, TS], BF16, tag=f"qf{mt}", name=f"qf{mt}")
        ft = ftiles[mt]
        for wi in range(4):
            nc.vector.scalar_tensor_tensor(
                qft[:, wi * 512:(wi + 1) * 512],
                cps[wi][:], 1.0,
                ft[:, wi * 512:(wi + 1) * 512],
                op0=AL.mult, op1=AL.mult)
        nc.scalar.activation(sptmp[:], qft[:], AF.Square,
                             accum_out=sqacc[:, mt * NT + t:mt * NT + t + 1])
        eng = (nc.sync, nc.scalar, nc.sync)[mt]
        eng.dma_start_transpose(
            qkfT[:].rearrange("p (k c) -> p k c", c=2 * C)
            [:, :, mt * 128:(mt + 1) * 128],
            qft[:])
    # gram over this tile's 16 chunks
    q3 = qkfT[:].rearrange("p (k c) -> p k c", c=2 * C)
    for ck in range(TR):
        g = t * TR + ck
        nc.tensor.matmul(
            g_ps[0][:], q3[:, ck, 0:128], q3[:, ck, C:2 * C],
            start=(g == 0), stop=(g == NT * TR - 1),
            skip_group_check=True)
        nc.tensor.matmul(
            g_ps[1][:], q3[:, ck, 128:192], q3[:, ck, C:2 * C],
            start=(g == 0), stop=(g == NT * TR - 1),
            skip_group_check=True)


def _conv_mms(nc, cps, xpk, st, j, first, last):
    for wi in range(4):
        yl = wi * 4
        base = (yl + 1) * PW + 1 + TAPD[j]
        mv = AP(xpk[:].tensor, xpk[:].offset + base,
                [[NPT, xpk.shape[0]], [PW, 4], [1, W]])
        nc.tensor.matmul(cps[wi][:], st, mv, start=first, stop=last,
                         skip_group_check=True)


def _v_stencil(nc, t, prev, v_sb, tapsv):
    """9-tap stencil for v channels 0:128, tile t (STT chain on DVE)."""
    vp = prev["vp"][t]
    vp3 = vp[:].rearrange("p (y x) -> p y x", x=PW)
    out = v_sb[:, t * TS:(t + 1) * TS].rearrange("p (y x) -> p y x", x=W)
    for j in range(9):
        dy, dx = divmod(j, 3)
        src = vp3[:, dy:dy + TR, dx:dx + W]
        if j == 0:
            nc.vector.tensor_scalar_mul(out, src, tapsv[:, j:j + 1])
        else:
            nc.vector.scalar_tensor_tensor(
                out, src, tapsv[:, j:j + 1], out, op0=AL.mult, op1=AL.add)


def _v_stencil_split(nc, vsplit, v_sp, tapsv, scrpool):
    """9-tap stencil for v-ch 128:192 in split layout (STT chain)."""
    vs3 = vsplit[:].rearrange("p (y x) -> p y x", x=PW)
    for c0 in range(0, 64, TR):
        out = v_sp[:, c0 * W:(c0 + TR) * W].rearrange("p (y x) -> p y x", x=W)
        for j in range(9):
            dy, dx = divmod(j, 3)
            src = vs3[:, c0 + dy:c0 + dy + TR, dx:dx + W]
            if j == 0:
                nc.vector.tensor_scalar_mul(out, src, tapsv[:, 9 + j:10 + j])
            else:
                nc.vector.scalar_tensor_tensor(
                    out, src, tapsv[:, 9 + j:10 + j], out,
                    op0=AL.mult, op1=AL.add)


def _host_prep(W_qkv, W_dw, W_proj, temperature):
    import ml_dtypes
    W1 = np.asarray(W_qkv, np.float32)          # [576, 192]
    taps = np.asarray(W_dw, np.float32).reshape(O, 9)
    W3 = taps[:, None, :] * W1[:, :, None]       # [576, 192, 9]
    # Mtiles: q0:128, q128:192||k0:64 -> outs 128:256, k64:192 -> 256:384
    w3 = np.zeros((128, 54 * 128), np.float32)
    for mt in range(3):
        mbase = mt * 128
        for kt, (cs, ksz) in enumerate(KT):
            for j in range(9):
                idx = (mt * 2 + kt) * 9 + j
                w3[0:ksz, idx * 128:(idx + 1) * 128] = \
                    W3[mbase:mbase + 128, cs:cs + ksz, j].T
    w3 = w3.astype(ml_dtypes.bfloat16)
    wv = np.ascontiguousarray(W1[384:576].T).astype(ml_dtypes.bfloat16)
    tapsv = np.zeros((128, 18), np.float32)
    tapsv[:, 0:9] = taps[384:512]
    tapsv[0:64, 9:18] = taps[512:576]
    tapsv[64:128, 9:18] = taps[512:576]
    wp = np.ascontiguousarray(np.asarray(W_proj, np.float32).T).astype(
        ml_dtypes.bfloat16)
    temp = np.broadcast_to(
        np.asarray(temperature, np.float32).reshape(1, HEADS), (CH, HEADS))
    return {
        "w3": w3, "wv": wv, "tapsv": tapsv, "wp": wp,
        "temp": np.ascontiguousarray(temp),
        "ident": np.eye(128, dtype=np.float32),
    }


_NC_CACHE = {}
TRACE = False
LAST = {}


def kernel(x, feature, W_qkv, W_dw, W_proj, temperature):
    b = x.shape[0]
    consts = _host_prep(W_qkv, W_dw, W_proj, temperature)

    if "nc" not in _NC_CACHE:
        _NC_CACHE["nc"] = build_nc()
    nc = _NC_CACHE["nc"]

    in_maps = []
    for i in range(b):
        m = {"x": np.ascontiguousarray(np.asarray(x[i], np.float32)),
             "f": np.ascontiguousarray(np.asarray(feature[i], np.float32))}
        m.update(consts)
        in_maps.append(m)
    res = run_bass_kernel_spmd(nc, in_maps, list(range(b)), trace=TRACE)
    LAST["exec_time_ns"] = res.exec_time_ns
    LAST["insts_trace"] = res.instructions_and_trace
    outs = [np.asarray(r["out"], np.float32).reshape(C, H, W)
            for r in res.results]
    return np.stack(outs, axis=0)
